# revision 29
# baseline (speedup 1.0000x reference)
"""Trainium2 Bass kernel for MHA with query-axis softmax (nn_MHA_2568390443327).

Reference computation (B=4, N=2048, DIM=1024, 16 heads x 64):
    qkv = x @ w_qkv ; q,k,v = split(qkv)
    scores = (q @ k^T) * scale            # [b,h,i(query),j(key)]
    attn = softmax(scores, axis=QUERY)    # normalized over i, per key j
    y = attn @ v ; out = y @ w_out + b_out

Sharding (8 cores): batch (4) x head-half (2). Each core gets its batch's
x (pre-transposed), the qkv weight columns and w_out rows for its 8 heads,
and produces a partial [DIM, N] output (transposed). Host sums the two
head-half partials per batch and transposes back.

Device schedule (v1): the kernel is activation-engine bound (33.5M exps
per core; ACT runs 1 elem/lane/cycle @1.2GHz). Scores are computed
transposed, S_T[j, i], in [128, 1024] fp32 PSUM slabs rotating through a
3-buffer pool (6 banks); the remaining 2 banks are a shared spare pool
for attn@v chunks and projection matmuls. Scores matmuls for unit u+1
are emitted ahead so the ACT queue never starves; both heads of a pair
run concurrently on row/col halves of the PE (full-array activity keeps
the HAM clock-gate warm). qkv/out projections are interleaved as filler
in the PE stream under the ACT shadow. The query-axis softmax denominator
comes free via the ACTIVATE accumulator; 1/den folds into a per-row
rescale of v. All matmul operands are float16 (fp32 PSUM accumulation).
"""

import os
import numpy as np

# ---------------------------------------------------------------------------
# Problem constants (hardcoded; kernel.py must be self-contained).
B = 4
N = 2048          # sequence length
F = 1024          # model dim (contraction for qkv proj)
HEADS_TOT = 16
DH = 64           # head dim
HH = 8            # heads per core (head-half)
CH = HH * DH      # 512: per-core hidden
OUT = 1024        # output dim
SCALE = 0.125     # 1/sqrt(64)
N_CORES = 8

P = 128           # partitions
NC512 = 512       # matmul free-dim chunk
SLAB_W = 1024     # ACT slab width (one per (pair, j, head, i-half))


def _build_nc():
    import concourse.bass as bass  # noqa: F401
    import concourse.mybir as mybir
    from concourse import bacc
    from concourse.tile import TileContext

    f32 = mybir.dt.float32
    f16 = mybir.dt.float16
    EXP = mybir.ActivationFunctionType.Exp

    nc = bacc.Bacc(None, target_bir_lowering=False)

    xT = nc.declare_dram_parameter("xT", [F, N], f16, isOutput=False)
    wqkv = nc.declare_dram_parameter("wqkv", [F, 3 * CH], f16, isOutput=False)
    wout = nc.declare_dram_parameter("wout", [CH, OUT], f16, isOutput=False)
    bias = nc.declare_dram_parameter("bias", [P, OUT // P], f32, isOutput=False)
    outT = nc.declare_dram_parameter("outT", [OUT, N], f32, isOutput=True)

    KT = F // P            # 8 k-tiles for the projections
    NT = N // P            # 16 j-tiles
    PAIRS = 4              # head pairs per core
    OT = OUT // P          # 8 output row tiles
    NCH = N // NC512       # 4 i-chunks of 512
    IH = N // SLAB_W       # 2 i-halves per slab row

    with TileContext(nc) as tc:
        with (
            tc.tile_pool(name="p_x", bufs=1) as p_x,
            tc.tile_pool(name="p_w", bufs=1) as p_w,
            tc.tile_pool(name="p_wout", bufs=1) as p_wout,
            tc.tile_pool(name="p_small", bufs=1) as p_small,
            tc.tile_pool(name="p_qk", bufs=2) as p_qk,
            tc.tile_pool(name="p_v", bufs=1) as p_v,
            tc.tile_pool(name="p_at", bufs=6) as p_at,
            tc.tile_pool(name="p_y", bufs=2) as p_y,
            tc.tile_pool(name="p_yf", bufs=1) as p_yf,
            tc.tile_pool(name="p_den", bufs=24) as p_den,
            tc.tile_pool(name="p_vp", bufs=8) as p_vp,
            tc.tile_pool(name="p_ti", bufs=2) as p_ti,
            tc.tile_pool(name="p_osb", bufs=2) as p_osb,
            tc.tile_pool(name="ps_slab", bufs=3, space="PSUM") as ps_slab,
            tc.tile_pool(name="ps_sp", bufs=2, space="PSUM") as ps_sp,
        ):
            # ---- input DMA + ACT exp-table warmup
            bias_sb = p_small.tile([P, OUT // P], f32, tag="bias",
                                   name="bias_sb")
            nc.sync.dma_start(out=bias_sb, in_=bias[:, :])
            warm = p_small.tile([P, OUT // P], f32, tag="warm", name="warm")
            nc.scalar.activation(warm, bias_sb, EXP)  # loads exp table early

            # PE warm-up: ~5us of full-array matmuls on scratch data during
            # the input DMA so the HAM clock-gate reaches 2.4 GHz before the
            # first projection burst (otherwise the prologue runs at 1.2).
            wsrc = p_small.tile([P, NC512], f16, tag="wsrc", name="wsrc")
            nc.vector.memset(wsrc, 0.0)
            wps = ps_sp.tile([P, NC512], f32, tag="sp", name="wps")
            for r in range(24):
                nc.tensor.matmul(wps, lhsT=wsrc[:, 0:P], rhs=wsrc,
                                 start=(r == 0), stop=(r == 23))

            xt = [p_x.tile([P, N], f16, tag=f"x{k}", name=f"x{k}")
                  for k in range(KT)]
            wt = [p_w.tile([P, 3 * CH], f16, tag=f"w{k}", name=f"w{k}")
                  for k in range(KT)]
            for k in range(KT):
                nc.sync.dma_start(out=xt[k], in_=xT[k * P:(k + 1) * P, :])
                nc.sync.dma_start(out=wt[k], in_=wqkv[k * P:(k + 1) * P, :])
            wout_sb = [p_wout.tile([P, OUT], f16, tag=f"wo{c}", name=f"wo{c}")
                       for c in range(PAIRS)]
            for c in range(PAIRS):
                nc.sync.dma_start(out=wout_sb[c],
                                  in_=wout[c * P:(c + 1) * P, :])

            v_sb = [p_v.tile([P, CH], f32, tag=f"v{j}", name=f"v{j}")
                    for j in range(NT)]
            y_f16 = [p_yf.tile([P, N], f16, tag=f"yf{pr}", name=f"yf{pr}")
                     for pr in range(PAIRS)]

            # ---- filler emitters (projections through the spare pool)
            def emit_v_unit(j):
                ps = ps_sp.tile([P, NC512], f32, tag="sp", name=f"psv{j}")
                for k in range(KT):
                    nc.tensor.matmul(
                        ps,
                        lhsT=xt[k][:, j * P:(j + 1) * P],
                        rhs=wt[k][:, 2 * CH:3 * CH],
                        start=(k == 0), stop=(k == KT - 1))
                nc.vector.tensor_copy(v_sb[j], ps)

            def emit_qk_chunk(dst, sec, pr, c):
                ps = ps_sp.tile([P, NC512], f32, tag="sp",
                                name=f"qk{sec}_{pr}_{c}")
                for k in range(KT):
                    nc.tensor.matmul(
                        ps,
                        lhsT=wt[k][:, sec * CH + pr * P:
                                   sec * CH + (pr + 1) * P],
                        rhs=xt[k][:, c * NC512:(c + 1) * NC512],
                        start=(k == 0), stop=(k == KT - 1))
                nc.vector.tensor_copy(dst[:, c * NC512:(c + 1) * NC512], ps)

            qk_tiles = {}

            def make_pair_fillers(pr):
                qT_t = p_qk.tile([P, N], f16, tag="q", name=f"qT{pr}")
                kT_t = p_qk.tile([P, N], f16, tag="k", name=f"kT{pr}")
                qk_tiles[pr] = (qT_t, kT_t)
                ops = []
                for sec, dst in ((0, qT_t), (1, kT_t)):
                    for c in range(NCH):
                        ops.append((emit_qk_chunk, dst, sec, pr, c))
                return ops

            # ---- attention unit emitters
            def emit_S(pr, j):
                """Score matmuls for unit (pr, j): both heads row-paired."""
                qT_t, kT_t = qk_tiles[pr]
                js = slice(j * P, (j + 1) * P)
                slabs = {}
                for ih in range(IH):
                    for h in (0, 1):
                        slabs[(ih, h)] = ps_slab.tile(
                            [P, SLAB_W], f32, tag="slab",
                            name=f"s{pr}_{j}_{ih}_{h}")
                    for c in range(SLAB_W // NC512):
                        i0 = ih * SLAB_W + c * NC512
                        for h in (0, 1):
                            nc.tensor.matmul(
                                slabs[(ih, h)][:, c * NC512:(c + 1) * NC512],
                                lhsT=kT_t[h * DH:(h + 1) * DH, js],
                                rhs=qT_t[h * DH:(h + 1) * DH, i0:i0 + NC512],
                                start=True, stop=True,
                                tile_position=(h * DH, 0))
                return {"pr": pr, "j": j, "slabs": slabs}

            # Schraudolph fast-exp constants: exp(s*SCALE) ~=
            # bitcast_f32(int32(A*s + B)); max rel err ~4%, rms ~2%.
            # Offloading every 6th slab (pairs 1-3) to DVE frees the
            # saturated scalar engine; the noise it adds to the output is
            # ~sqrt(32/256)*2% ~ 0.7% against a 2e-2 gate.
            SCH_A = float((1 << 23) * 1.4426950408889634 * SCALE)
            SCH_B = 1064866805.0
            MULT = mybir.AluOpType.mult
            ADD = mybir.AluOpType.add
            i32 = mybir.dt.int32
            slab_ctr = [0]

            def emit_ACT(st):
                pr, j = st["pr"], st["j"]
                at = {h: p_at.tile([P, N], f16, tag="at",
                                   name=f"at{pr}_{j}_{h}") for h in (0, 1)}
                dens = {}
                for ih in range(IH):
                    for h in (0, 1):
                        den = p_den.tile([P, 1], f32, tag="den",
                                         name=f"d{pr}_{j}_{ih}_{h}")
                        dst = at[h][:, ih * SLAB_W:(ih + 1) * SLAB_W]
                        slab_ctr[0] += 1
                        if pr >= 1 and slab_ctr[0] % 6 == 0:
                            ti = p_ti.tile([P, SLAB_W], i32, tag="ti",
                                           name="ti")
                            nc.vector.tensor_scalar(
                                ti, st["slabs"][(ih, h)], SCH_A, SCH_B,
                                MULT, ADD)
                            nc.vector.tensor_scalar(
                                dst, ti.bitcast(f32), 1.0, None, MULT, ADD,
                                accum_out=den)
                        else:
                            nc.scalar.activation(
                                dst, st["slabs"][(ih, h)], EXP,
                                scale=SCALE, accum_out=den)
                        dens[(ih, h)] = den
                st["at"], st["dens"] = at, dens

            def emit_vp(st):
                pr, j = st["pr"], st["j"]
                vps = {}
                for h in (0, 1):
                    dtot = p_den.tile([P, 1], f32, tag="den", name="dtot")
                    nc.vector.tensor_add(dtot, st["dens"][(0, h)],
                                         st["dens"][(1, h)])
                    rec = p_den.tile([P, 1], f32, tag="den", name="rec")
                    nc.vector.reciprocal(rec, dtot)
                    vp = p_vp.tile([P, DH], f16, tag="vp", name=f"vp{h}")
                    c0 = pr * 2 * DH + h * DH
                    nc.vector.tensor_scalar_mul(
                        vp, v_sb[j][:, c0:c0 + DH], rec)
                    vps[h] = vp
                st["vps"] = vps

            def emit_y(st, y_cur):
                j, at, vps = st["j"], st["at"], st["vps"]
                for c in range(NCH):
                    ych = ps_sp.tile([P, NC512], f32, tag="sp",
                                     name=f"ych{c}")
                    for h in (0, 1):
                        nc.tensor.matmul(
                            ych[h * DH:(h + 1) * DH, :],
                            lhsT=vps[h],
                            rhs=at[h][:, c * NC512:(c + 1) * NC512],
                            start=True, stop=True,
                            tile_position=(0, h * DH))
                    ysl = y_cur[:, c * NC512:(c + 1) * NC512]
                    if j == 0:
                        nc.vector.tensor_copy(ysl, ych)
                    else:
                        nc.vector.tensor_add(ysl, ych, ysl)

            # ---- prologue: q/k for pair 0 paced by the input DMA stream
            # (k-outer over 8 chunk positions spread across ps_sp + the
            # slab pool so every k-step issues as soon as x[k]/w[k] land),
            # then v[0:2] and the first score matmuls.
            fillers = []
            qT0 = p_qk.tile([P, N], f16, tag="q", name="qT0")
            kT0 = p_qk.tile([P, N], f16, tag="k", name="kT0")
            qk_tiles[0] = (qT0, kT0)
            pro_ps = [ps_sp.tile([P, NC512], f32, tag="sp", name=f"pp{i}")
                      for i in range(2)]
            pro_sl = [ps_slab.tile([P, SLAB_W], f32, tag="slab",
                                   name=f"psl{i}") for i in range(3)]
            chunk_ps = pro_ps + [pro_sl[i // 2][:, (i % 2) * NC512:
                                                 (i % 2 + 1) * NC512]
                                 for i in range(6)]
            for k in range(KT):
                for ch in range(8):
                    sec, c = divmod(ch, NCH)
                    nc.tensor.matmul(
                        chunk_ps[ch],
                        lhsT=wt[k][:, sec * CH + 0 * P:sec * CH + 1 * P],
                        rhs=xt[k][:, c * NC512:(c + 1) * NC512],
                        start=(k == 0), stop=(k == KT - 1))
            for ch in range(8):
                sec, c = divmod(ch, NCH)
                dst = qT0 if sec == 0 else kT0
                nc.vector.tensor_copy(
                    dst[:, c * NC512:(c + 1) * NC512], chunk_ps[ch])
            emit_v_unit(0)
            emit_v_unit(1)

            state = emit_S(0, 0)
            # v[2:8] run in the PE shadow of the first ACTIVATEs; the rest
            # pace through pair 0 as fillers.
            for j in range(2, 8):
                emit_v_unit(j)
            for j in range(8, NT):
                fillers.append((emit_v_unit, j))
            fillers.extend(make_pair_fillers(1))
            y_cur = None

            # ---- main software-pipelined loop over 64 units
            for u in range(PAIRS * NT):
                pr, j = divmod(u, NT)
                if j == 0:
                    y_cur = p_y.tile([P, N], f32, tag="y", name=f"y{pr}")
                nxt = None
                if u + 1 < PAIRS * NT:
                    nxt = emit_S((u + 1) // NT, (u + 1) % NT)
                emit_ACT(state)
                emit_vp(state)
                emit_y(state, y_cur)
                if j == NT - 1:
                    nc.vector.tensor_copy(y_f16[pr], y_cur)
                # pace fillers: v units + next pair's qk in pair 0; next
                # pair's qk in pairs 1-2; out-proj partials in pair 3.
                quota = 2 if (pr == 0 and j >= 8) else 1
                for _ in range(quota):
                    if not fillers:
                        break
                    op = fillers.pop(0)
                    op[0](*op[1:])
                if j == 0 and pr + 1 < PAIRS and pr >= 1:
                    fillers.extend(make_pair_fillers(pr + 1))
                state = nxt

            # ---- epilogue: output projection. The slab banks are free
            # after the last ACTIVATE, so spread the chunk pipeline over 8
            # PSUM slots; batch the DMA per o-tile row block.
            ep_sl = [ps_slab.tile([P, SLAB_W], f32, tag="slab",
                                  name=f"esl{i}") for i in range(3)]
            ep_ps = ([ps_sp.tile([P, NC512], f32, tag="sp", name=f"ep{i}")
                      for i in range(2)]
                     + [ep_sl[i // 2][:, (i % 2) * NC512:
                                      (i % 2 + 1) * NC512]
                        for i in range(6)])
            IDENT = mybir.ActivationFunctionType.Identity
            for o in range(OT):
                osb = p_osb.tile([P, N], f32, tag="osb", name=f"osb{o}")
                for ich in range(NCH):
                    ps = ep_ps[(o * NCH + ich) % 8]
                    for c in range(PAIRS):
                        nc.tensor.matmul(
                            ps,
                            lhsT=wout_sb[c][:, o * P:(o + 1) * P],
                            rhs=y_f16[c][:, ich * NC512:(ich + 1) * NC512],
                            start=(c == 0), stop=(c == PAIRS - 1))
                    dst = osb[:, ich * NC512:(ich + 1) * NC512]
                    # split the PSUM->SBUF bias-add between DVE and the
                    # (idle) scalar engine so the tail is not DVE-serial
                    if ich % 2 == 0:
                        nc.vector.tensor_scalar_add(dst, ps,
                                                    bias_sb[:, o:o + 1])
                    else:
                        nc.scalar.activation(dst, ps, IDENT,
                                             bias=bias_sb[:, o:o + 1])
                nc.sync.dma_start(out=outT[o * P:(o + 1) * P, :], in_=osb)
    return nc


def _shard_inputs(x, w_qkv, w_out, b_out):
    """Build per-core input maps: core c -> (batch c//2, head-half c%2)."""
    in_maps = []
    for c in range(N_CORES):
        b, hh = c // 2, c % 2
        cols = slice(hh * CH, (hh + 1) * CH)
        xTc = np.ascontiguousarray(np.asarray(x[b]).T, dtype=np.float16)
        wq = w_qkv[:, 0 * F:1 * F][:, cols]
        wk = w_qkv[:, 1 * F:2 * F][:, cols]
        wv = w_qkv[:, 2 * F:3 * F][:, cols]
        wqkv_c = np.ascontiguousarray(
            np.concatenate([wq, wk, wv], axis=1), dtype=np.float16)
        wout_c = np.ascontiguousarray(w_out[cols, :], dtype=np.float16)
        bias_c = np.ascontiguousarray(
            (np.asarray(b_out, dtype=np.float32) / 2.0)
            .reshape(OUT // P, P).T)
        in_maps.append(
            {"xT": xTc, "wqkv": wqkv_c, "wout": wout_c, "bias": bias_c})
    return in_maps


def _gather_outputs(results):
    out = np.empty((B, N, OUT), np.float32)
    for b in range(B):
        acc = results[2 * b]["outT"] + results[2 * b + 1]["outT"]  # [OUT, N]
        out[b] = acc.T
    return out


# Test instrumentation (harness just calls kernel(); these stay default).
_TRACE = False
_LAST_RESULT = None


def kernel(x, w_qkv, w_out, b_out):
    global _LAST_RESULT
    # The bass->PJRT path needs the axon trn2 devices visible to jax.
    if os.environ.get("JAX_PLATFORMS") not in (None, "", "axon"):
        os.environ.pop("JAX_PLATFORMS", None)
    from concourse.bass_utils import run_bass_kernel_spmd

    nc = _build_nc()
    if not nc.is_finalized():
        nc.finalize()  # runs Bacc legalization (wait splitting, reg alloc)
    in_maps = _shard_inputs(np.asarray(x), np.asarray(w_qkv),
                            np.asarray(w_out), np.asarray(b_out))
    res = run_bass_kernel_spmd(nc, in_maps, list(range(N_CORES)),
                               trace=_TRACE)
    _LAST_RESULT = res
    return _gather_outputs(res.results)


# ---------------------------------------------------------------------------
# Numpy emulation of the per-core device program (for host-logic testing;
# not used by kernel()).
def _emulate_core(m):
    xT, wqkv, wout, bias = m["xT"], m["wqkv"], m["wout"], m["bias"]
    qT = (wqkv[:, 0:CH].T @ xT)          # [CH, N]
    kTm = (wqkv[:, CH:2 * CH].T @ xT)    # [CH, N]
    v = xT.T @ wqkv[:, 2 * CH:3 * CH]    # [N, CH]
    outT_acc = np.zeros((OUT, N), np.float32)
    y = np.empty((CH, N), np.float32)
    for h in range(HH):
        qh = qT[h * DH:(h + 1) * DH, :]      # [DH, N(i)]
        kh = kTm[h * DH:(h + 1) * DH, :]     # [DH, N(j)]
        sT = kh.T @ qh                       # [j, i]
        e = np.exp(sT * SCALE)
        den = e.sum(axis=1, keepdims=True)   # over queries i, per key j
        vp = v[:, h * DH:(h + 1) * DH] / den
        y[h * DH:(h + 1) * DH, :] = vp.T @ e  # [DH, i]
    outT_acc = wout.T @ y                    # [OUT, N]
    outT_acc += bias.T.reshape(OUT, 1)
    return {"outT": outT_acc}


def _kernel_emulated(x, w_qkv, w_out, b_out):
    in_maps = _shard_inputs(np.asarray(x), np.asarray(w_qkv),
                            np.asarray(w_out), np.asarray(b_out))
    results = [_emulate_core(m) for m in in_maps]
    return _gather_outputs(results)


# revision 30
# speedup vs baseline: 1.1259x; 1.1259x over previous
"""Trainium2 Bass kernel for MHA with query-axis softmax (nn_MHA_2568390443327).

Reference computation (B=4, N=2048, DIM=1024, 16 heads x 64):
    qkv = x @ w_qkv ; q,k,v = split(qkv)
    scores = (q @ k^T) * scale            # [b,h,i(query),j(key)]
    attn = softmax(scores, axis=QUERY)    # normalized over i, per key j
    y = attn @ v ; out = y @ w_out + b_out

Sharding (8 cores): batch (4) x head-half (2). Each core gets its batch's
x (pre-transposed), the qkv weight columns and w_out rows for its 8 heads,
and produces a partial [DIM, N] output (transposed). Host sums the two
head-half partials per batch and transposes back.

Device schedule (v1): the kernel is activation-engine bound (33.5M exps
per core; ACT runs 1 elem/lane/cycle @1.2GHz). Scores are computed
transposed, S_T[j, i], in [128, 1024] fp32 PSUM slabs rotating through a
3-buffer pool (6 banks); the remaining 2 banks are a shared spare pool
for attn@v chunks and projection matmuls. Scores matmuls for unit u+1
are emitted ahead so the ACT queue never starves; both heads of a pair
run concurrently on row/col halves of the PE (full-array activity keeps
the HAM clock-gate warm). qkv/out projections are interleaved as filler
in the PE stream under the ACT shadow. The query-axis softmax denominator
comes free via the ACTIVATE accumulator; 1/den folds into a per-row
rescale of v. All matmul operands are float16 (fp32 PSUM accumulation).
"""

import os
import numpy as np

# ---------------------------------------------------------------------------
# Problem constants (hardcoded; kernel.py must be self-contained).
B = 4
N = 2048          # sequence length
F = 1024          # model dim (contraction for qkv proj)
HEADS_TOT = 16
DH = 64           # head dim
HH = 8            # heads per core (head-half)
CH = HH * DH      # 512: per-core hidden
OUT = 1024        # output dim
SCALE = 0.125     # 1/sqrt(64)
N_CORES = 8

P = 128           # partitions
NC512 = 512       # matmul free-dim chunk
SLAB_W = 1024     # ACT slab width (one per (pair, j, head, i-half))


def _build_nc():
    import concourse.bass as bass  # noqa: F401
    import concourse.mybir as mybir
    from concourse import bacc
    from concourse.tile import TileContext

    f32 = mybir.dt.float32
    f16 = mybir.dt.float16
    EXP = mybir.ActivationFunctionType.Exp

    nc = bacc.Bacc(None, target_bir_lowering=False)

    xT = nc.declare_dram_parameter("xT", [F, N], f16, isOutput=False)
    wqkv = nc.declare_dram_parameter("wqkv", [F, 3 * CH], f16, isOutput=False)
    wout = nc.declare_dram_parameter("wout", [CH, OUT], f16, isOutput=False)
    bias = nc.declare_dram_parameter("bias", [P, OUT // P], f32, isOutput=False)
    outT = nc.declare_dram_parameter("outT", [OUT, N], f32, isOutput=True)

    KT = F // P            # 8 k-tiles for the projections
    NT = N // P            # 16 j-tiles
    PAIRS = 4              # head pairs per core
    OT = OUT // P          # 8 output row tiles
    NCH = N // NC512       # 4 i-chunks of 512
    IH = N // SLAB_W       # 2 i-halves per slab row

    with TileContext(nc) as tc:
        with (
            tc.tile_pool(name="p_x", bufs=1) as p_x,
            tc.tile_pool(name="p_w", bufs=1) as p_w,
            tc.tile_pool(name="p_wout", bufs=1) as p_wout,
            tc.tile_pool(name="p_small", bufs=1) as p_small,
            tc.tile_pool(name="p_qk", bufs=2) as p_qk,
            tc.tile_pool(name="p_v", bufs=1) as p_v,
            tc.tile_pool(name="p_at", bufs=6) as p_at,
            tc.tile_pool(name="p_y", bufs=2) as p_y,
            tc.tile_pool(name="p_yf", bufs=1) as p_yf,
            tc.tile_pool(name="p_den", bufs=24) as p_den,
            tc.tile_pool(name="p_vp", bufs=8) as p_vp,
            tc.tile_pool(name="p_ti", bufs=2) as p_ti,
            tc.tile_pool(name="p_osb", bufs=2) as p_osb,
            tc.tile_pool(name="ps_slab", bufs=3, space="PSUM") as ps_slab,
            tc.tile_pool(name="ps_sp", bufs=2, space="PSUM") as ps_sp,
        ):
            # ---- input DMA + ACT exp-table warmup
            bias_sb = p_small.tile([P, OUT // P], f32, tag="bias",
                                   name="bias_sb")
            nc.sync.dma_start(out=bias_sb, in_=bias[:, :])
            warm = p_small.tile([P, OUT // P], f32, tag="warm", name="warm")
            nc.scalar.activation(warm, bias_sb, EXP)  # loads exp table early

            # PE warm-up: ~5us of full-array matmuls on scratch data during
            # the input DMA so the HAM clock-gate reaches 2.4 GHz before the
            # first projection burst (otherwise the prologue runs at 1.2).
            wsrc = p_small.tile([P, NC512], f16, tag="wsrc", name="wsrc")
            nc.vector.memset(wsrc, 0.0)
            wps = ps_sp.tile([P, NC512], f32, tag="sp", name="wps")
            for r in range(24):
                nc.tensor.matmul(wps, lhsT=wsrc[:, 0:P], rhs=wsrc,
                                 start=(r == 0), stop=(r == 23))

            xt = [p_x.tile([P, N], f16, tag=f"x{k}", name=f"x{k}")
                  for k in range(KT)]
            wt = [p_w.tile([P, 3 * CH], f16, tag=f"w{k}", name=f"w{k}")
                  for k in range(KT)]
            for k in range(KT):
                nc.sync.dma_start(out=xt[k], in_=xT[k * P:(k + 1) * P, :])
                nc.sync.dma_start(out=wt[k], in_=wqkv[k * P:(k + 1) * P, :])
            wout_sb = [p_wout.tile([P, OUT], f16, tag=f"wo{c}", name=f"wo{c}")
                       for c in range(PAIRS)]
            for c in range(PAIRS):
                nc.sync.dma_start(out=wout_sb[c],
                                  in_=wout[c * P:(c + 1) * P, :])

            v_sb = [p_v.tile([P, CH], f32, tag=f"v{j}", name=f"v{j}")
                    for j in range(NT)]
            y_f16 = [p_yf.tile([P, N], f16, tag=f"yf{pr}", name=f"yf{pr}")
                     for pr in range(PAIRS)]

            # ---- filler emitters (projections through the spare pool)
            def emit_v_unit(j):
                ps = ps_sp.tile([P, NC512], f32, tag="sp", name=f"psv{j}")
                for k in range(KT):
                    nc.tensor.matmul(
                        ps,
                        lhsT=xt[k][:, j * P:(j + 1) * P],
                        rhs=wt[k][:, 2 * CH:3 * CH],
                        start=(k == 0), stop=(k == KT - 1))
                nc.vector.tensor_copy(v_sb[j], ps)

            def emit_qk_chunk(dst, sec, pr, c):
                ps = ps_sp.tile([P, NC512], f32, tag="sp",
                                name=f"qk{sec}_{pr}_{c}")
                for k in range(KT):
                    nc.tensor.matmul(
                        ps,
                        lhsT=wt[k][:, sec * CH + pr * P:
                                   sec * CH + (pr + 1) * P],
                        rhs=xt[k][:, c * NC512:(c + 1) * NC512],
                        start=(k == 0), stop=(k == KT - 1))
                nc.vector.tensor_copy(dst[:, c * NC512:(c + 1) * NC512], ps)

            qk_tiles = {}

            def make_pair_fillers(pr):
                qT_t = p_qk.tile([P, N], f16, tag="q", name=f"qT{pr}")
                kT_t = p_qk.tile([P, N], f16, tag="k", name=f"kT{pr}")
                qk_tiles[pr] = (qT_t, kT_t)
                ops = []
                for sec, dst in ((0, qT_t), (1, kT_t)):
                    for c in range(NCH):
                        ops.append((emit_qk_chunk, dst, sec, pr, c))
                return ops

            # ---- attention unit emitters
            def emit_S(pr, j):
                """Score matmuls for unit (pr, j): both heads row-paired."""
                qT_t, kT_t = qk_tiles[pr]
                js = slice(j * P, (j + 1) * P)
                slabs = {}
                for ih in range(IH):
                    for h in (0, 1):
                        slabs[(ih, h)] = ps_slab.tile(
                            [P, SLAB_W], f32, tag="slab",
                            name=f"s{pr}_{j}_{ih}_{h}")
                    for c in range(SLAB_W // NC512):
                        i0 = ih * SLAB_W + c * NC512
                        for h in (0, 1):
                            nc.tensor.matmul(
                                slabs[(ih, h)][:, c * NC512:(c + 1) * NC512],
                                lhsT=kT_t[h * DH:(h + 1) * DH, js],
                                rhs=qT_t[h * DH:(h + 1) * DH, i0:i0 + NC512],
                                start=True, stop=True,
                                tile_position=(h * DH, 0))
                return {"pr": pr, "j": j, "slabs": slabs}

            # Schraudolph fast-exp constants: exp(s*SCALE) ~=
            # bitcast_f32(int32(A*s + B)); max rel err ~4%, rms ~2%.
            # Offloading every 6th slab (pairs 1-3) to DVE frees the
            # saturated scalar engine; the noise it adds to the output is
            # ~sqrt(32/256)*2% ~ 0.7% against a 2e-2 gate.
            SCH_A = float((1 << 23) * 1.4426950408889634 * SCALE)
            SCH_B = 1064866805.0
            MULT = mybir.AluOpType.mult
            ADD = mybir.AluOpType.add
            i32 = mybir.dt.int32
            slab_ctr = [0]

            def emit_ACT(st):
                pr, j = st["pr"], st["j"]
                at = {h: p_at.tile([P, N], f16, tag="at",
                                   name=f"at{pr}_{j}_{h}") for h in (0, 1)}
                dens = {}
                for ih in range(IH):
                    for h in (0, 1):
                        den = p_den.tile([P, 1], f32, tag="den",
                                         name=f"d{pr}_{j}_{ih}_{h}")
                        dst = at[h][:, ih * SLAB_W:(ih + 1) * SLAB_W]
                        slab_ctr[0] += 1
                        nc.scalar.activation(
                            dst, st["slabs"][(ih, h)], EXP,
                            scale=SCALE, accum_out=den)
                        dens[(ih, h)] = den
                st["at"], st["dens"] = at, dens

            def emit_vp(st):
                pr, j = st["pr"], st["j"]
                vps = {}
                for h in (0, 1):
                    dtot = p_den.tile([P, 1], f32, tag="den", name="dtot")
                    nc.vector.tensor_add(dtot, st["dens"][(0, h)],
                                         st["dens"][(1, h)])
                    rec = p_den.tile([P, 1], f32, tag="den", name="rec")
                    nc.vector.reciprocal(rec, dtot)
                    vp = p_vp.tile([P, DH], f16, tag="vp", name=f"vp{h}")
                    c0 = pr * 2 * DH + h * DH
                    nc.vector.tensor_scalar_mul(
                        vp, v_sb[j][:, c0:c0 + DH], rec)
                    vps[h] = vp
                st["vps"] = vps

            def emit_y(st, y_cur):
                j, at, vps = st["j"], st["at"], st["vps"]
                for c in range(NCH):
                    ych = ps_sp.tile([P, NC512], f32, tag="sp",
                                     name=f"ych{c}")
                    for h in (0, 1):
                        nc.tensor.matmul(
                            ych[h * DH:(h + 1) * DH, :],
                            lhsT=vps[h],
                            rhs=at[h][:, c * NC512:(c + 1) * NC512],
                            start=True, stop=True,
                            tile_position=(0, h * DH))
                    ysl = y_cur[:, c * NC512:(c + 1) * NC512]
                    if j == 0:
                        nc.vector.tensor_copy(ysl, ych)
                    else:
                        nc.vector.tensor_add(ysl, ych, ysl)

            # ---- prologue: q/k for pair 0 paced by the input DMA stream
            # (k-outer over 8 chunk positions spread across ps_sp + the
            # slab pool so every k-step issues as soon as x[k]/w[k] land),
            # then v[0:2] and the first score matmuls.
            fillers = []
            qT0 = p_qk.tile([P, N], f16, tag="q", name="qT0")
            kT0 = p_qk.tile([P, N], f16, tag="k", name="kT0")
            qk_tiles[0] = (qT0, kT0)
            pro_ps = [ps_sp.tile([P, NC512], f32, tag="sp", name=f"pp{i}")
                      for i in range(2)]
            pro_sl = [ps_slab.tile([P, SLAB_W], f32, tag="slab",
                                   name=f"psl{i}") for i in range(3)]
            chunk_ps = pro_ps + [pro_sl[i // 2][:, (i % 2) * NC512:
                                                 (i % 2 + 1) * NC512]
                                 for i in range(6)]
            for k in range(KT):
                for ch in range(8):
                    sec, c = divmod(ch, NCH)
                    nc.tensor.matmul(
                        chunk_ps[ch],
                        lhsT=wt[k][:, sec * CH + 0 * P:sec * CH + 1 * P],
                        rhs=xt[k][:, c * NC512:(c + 1) * NC512],
                        start=(k == 0), stop=(k == KT - 1))
            for ch in range(8):
                sec, c = divmod(ch, NCH)
                dst = qT0 if sec == 0 else kT0
                nc.vector.tensor_copy(
                    dst[:, c * NC512:(c + 1) * NC512], chunk_ps[ch])
            emit_v_unit(0)
            emit_v_unit(1)

            state = emit_S(0, 0)
            # v[2:8] run in the PE shadow of the first ACTIVATEs; the rest
            # pace through pair 0 as fillers.
            for j in range(2, 8):
                emit_v_unit(j)
            for j in range(8, NT):
                fillers.append((emit_v_unit, j))
            fillers.extend(make_pair_fillers(1))
            y_cur = None

            # ---- main software-pipelined loop over 64 units
            for u in range(PAIRS * NT):
                pr, j = divmod(u, NT)
                if j == 0:
                    y_cur = p_y.tile([P, N], f32, tag="y", name=f"y{pr}")
                nxt = None
                if u + 1 < PAIRS * NT:
                    nxt = emit_S((u + 1) // NT, (u + 1) % NT)
                emit_ACT(state)
                emit_vp(state)
                emit_y(state, y_cur)
                if j == NT - 1:
                    nc.vector.tensor_copy(y_f16[pr], y_cur)
                # pace fillers: v units + next pair's qk in pair 0; next
                # pair's qk in pairs 1-2; out-proj partials in pair 3.
                quota = 2 if (pr == 0 and j >= 8) else 1
                for _ in range(quota):
                    if not fillers:
                        break
                    op = fillers.pop(0)
                    op[0](*op[1:])
                if j == 0 and pr + 1 < PAIRS and pr >= 1:
                    fillers.extend(make_pair_fillers(pr + 1))
                state = nxt

            # ---- epilogue: output projection. The slab banks are free
            # after the last ACTIVATE, so spread the chunk pipeline over 8
            # PSUM slots; batch the DMA per o-tile row block.
            ep_sl = [ps_slab.tile([P, SLAB_W], f32, tag="slab",
                                  name=f"esl{i}") for i in range(3)]
            ep_ps = ([ps_sp.tile([P, NC512], f32, tag="sp", name=f"ep{i}")
                      for i in range(2)]
                     + [ep_sl[i // 2][:, (i % 2) * NC512:
                                      (i % 2 + 1) * NC512]
                        for i in range(6)])
            IDENT = mybir.ActivationFunctionType.Identity
            for o in range(OT):
                osb = p_osb.tile([P, N], f32, tag="osb", name=f"osb{o}")
                for ich in range(NCH):
                    ps = ep_ps[(o * NCH + ich) % 8]
                    for c in range(PAIRS):
                        nc.tensor.matmul(
                            ps,
                            lhsT=wout_sb[c][:, o * P:(o + 1) * P],
                            rhs=y_f16[c][:, ich * NC512:(ich + 1) * NC512],
                            start=(c == 0), stop=(c == PAIRS - 1))
                    dst = osb[:, ich * NC512:(ich + 1) * NC512]
                    # split the PSUM->SBUF bias-add between DVE and the
                    # (idle) scalar engine so the tail is not DVE-serial
                    if ich % 2 == 0:
                        nc.vector.tensor_scalar_add(dst, ps,
                                                    bias_sb[:, o:o + 1])
                    else:
                        nc.scalar.activation(dst, ps, IDENT,
                                             bias=bias_sb[:, o:o + 1])
                nc.sync.dma_start(out=outT[o * P:(o + 1) * P, :], in_=osb)
    return nc


def _shard_inputs(x, w_qkv, w_out, b_out):
    """Build per-core input maps: core c -> (batch c//2, head-half c%2)."""
    in_maps = []
    for c in range(N_CORES):
        b, hh = c // 2, c % 2
        cols = slice(hh * CH, (hh + 1) * CH)
        xTc = np.ascontiguousarray(np.asarray(x[b]).T, dtype=np.float16)
        wq = w_qkv[:, 0 * F:1 * F][:, cols]
        wk = w_qkv[:, 1 * F:2 * F][:, cols]
        wv = w_qkv[:, 2 * F:3 * F][:, cols]
        wqkv_c = np.ascontiguousarray(
            np.concatenate([wq, wk, wv], axis=1), dtype=np.float16)
        wout_c = np.ascontiguousarray(w_out[cols, :], dtype=np.float16)
        bias_c = np.ascontiguousarray(
            (np.asarray(b_out, dtype=np.float32) / 2.0)
            .reshape(OUT // P, P).T)
        in_maps.append(
            {"xT": xTc, "wqkv": wqkv_c, "wout": wout_c, "bias": bias_c})
    return in_maps


def _gather_outputs(results):
    out = np.empty((B, N, OUT), np.float32)
    for b in range(B):
        acc = results[2 * b]["outT"] + results[2 * b + 1]["outT"]  # [OUT, N]
        out[b] = acc.T
    return out


# Test instrumentation (harness just calls kernel(); these stay default).
_TRACE = False
_LAST_RESULT = None


def kernel(x, w_qkv, w_out, b_out):
    global _LAST_RESULT
    # The bass->PJRT path needs the axon trn2 devices visible to jax.
    if os.environ.get("JAX_PLATFORMS") not in (None, "", "axon"):
        os.environ.pop("JAX_PLATFORMS", None)
    from concourse.bass_utils import run_bass_kernel_spmd

    nc = _build_nc()
    if not nc.is_finalized():
        nc.finalize()  # runs Bacc legalization (wait splitting, reg alloc)
    in_maps = _shard_inputs(np.asarray(x), np.asarray(w_qkv),
                            np.asarray(w_out), np.asarray(b_out))
    res = run_bass_kernel_spmd(nc, in_maps, list(range(N_CORES)),
                               trace=_TRACE)
    _LAST_RESULT = res
    return _gather_outputs(res.results)


# ---------------------------------------------------------------------------
# Numpy emulation of the per-core device program (for host-logic testing;
# not used by kernel()).
def _emulate_core(m):
    xT, wqkv, wout, bias = m["xT"], m["wqkv"], m["wout"], m["bias"]
    qT = (wqkv[:, 0:CH].T @ xT)          # [CH, N]
    kTm = (wqkv[:, CH:2 * CH].T @ xT)    # [CH, N]
    v = xT.T @ wqkv[:, 2 * CH:3 * CH]    # [N, CH]
    outT_acc = np.zeros((OUT, N), np.float32)
    y = np.empty((CH, N), np.float32)
    for h in range(HH):
        qh = qT[h * DH:(h + 1) * DH, :]      # [DH, N(i)]
        kh = kTm[h * DH:(h + 1) * DH, :]     # [DH, N(j)]
        sT = kh.T @ qh                       # [j, i]
        e = np.exp(sT * SCALE)
        den = e.sum(axis=1, keepdims=True)   # over queries i, per key j
        vp = v[:, h * DH:(h + 1) * DH] / den
        y[h * DH:(h + 1) * DH, :] = vp.T @ e  # [DH, i]
    outT_acc = wout.T @ y                    # [OUT, N]
    outT_acc += bias.T.reshape(OUT, 1)
    return {"outT": outT_acc}


def _kernel_emulated(x, w_qkv, w_out, b_out):
    in_maps = _shard_inputs(np.asarray(x), np.asarray(w_qkv),
                            np.asarray(w_out), np.asarray(b_out))
    results = [_emulate_core(m) for m in in_maps]
    return _gather_outputs(results)
